# revision 10
# baseline (speedup 1.0000x reference)
"""Trainium2 Bass kernel for nn_Encoder_25847113187503 (4-block Mamba encoder).

Self-contained: hardcodes shapes from the problem spec.
  x: [4, 32, 128, 256] f32 -> out: [5, 4, 32, 128, 256] f32

out = stack([x, f1(x), f2(mix2(f1)), f3(mix3(f1)), f4(mix4(f1))]) where
fK = _fdse(.; params[bK]) = time-mammlp over cyc (L=128) then dim-mammlp over
tsd (L=32), and mixK adds deterministic jax threefry noise.

Device strategy (8 NeuronCores, SPMD): two compiled programs — T-kernel
(16 seqs x L=128) and D-kernel (64 x 32) — each processing 2048 tokens/core
of one mammlp in layout [feature-on-partition, token-on-free]. 8 launches:
L1: b1.time, L2: b1.dim, L3a-c: bK.time, L4a-c: bK.dim. Host does the cheap
column permutations between launches; exact 1/8 sharding per core, no
redundancy, no collectives.

Mamba internals on device:
  - selective scan via hw tensor_tensor_scan (state = a*state + b along the
    free dim), one (d, n, seq) segment chain per partition row; segment
    resets are a=0 at each sequence start (state := b).
  - decay a = exp(-n*dt) via ACT Exp with per-call scale=-n (A[d,n] = -n
    exactly, from A_log = log(tile(arange(1,17)))).
  - dt = softplus(v) = Ln(Exp(v) + 1)   (no softplus LUT on trn2).
  - silu(x) = x * Sigmoid(x)            (identical on CoreSim and hw).
  - B/C broadcast across partitions via a DRAM bounce (DRAM reads can
    replicate a row to 128 partitions; SBUF APs cannot).
  - all matmuls in fp16 operands (full PE rate), fp32 PSUM accumulate;
    residual/norm arithmetic in fp32.
"""
import sys
import os

if "/opt/trn_rl_repo" not in sys.path:
    sys.path.insert(0, "/opt/trn_rl_repo")
os.environ.setdefault("JAX_PLATFORMS", "cpu")

import numpy as np

B_, TSD, CYC, DM = 4, 32, 128, 256
DI = 512          # d_inner
NST = 16          # d_state
DTR = 16          # dt_rank
DFF = 1024
NCORES = 8
T = 2048          # tokens per core per launch
CH = 512          # matmul column chunk
TCH = 1024        # grid-phase column chunk
NB = 4            # d_state values per grid block

# gelu on sim: sigmoid approx (CoreSim lacks the Gelu LUT); on hw: exact
SIM_GELU = False


# ---------------------------------------------------------------- builder --
def build_program(n_seq, L, sim_gelu=False):
    import concourse.mybir as mybir
    from concourse import bacc, tile
    from contextlib import ExitStack

    f32 = mybir.dt.float32
    f16 = mybir.dt.float16
    AF = mybir.ActivationFunctionType
    MUL = mybir.AluOpType.mult
    ADD = mybir.AluOpType.add

    assert n_seq * L == T

    nc = bacc.Bacc("TRN2", target_bir_lowering=False, debug=False,
                   num_devices=NCORES)

    # ---- DRAM I/O ----
    xin_d = nc.dram_tensor("xin", [DM, T], f16, kind="ExternalInput")
    w_in_d = nc.dram_tensor("w_in", [DM, DFF], f16, kind="ExternalInput")
    w_xp_d = nc.dram_tensor("w_xp", [128, 4, 48], f16, kind="ExternalInput")
    w_dtw_d = nc.dram_tensor("w_dtw", [DTR, DI], f16, kind="ExternalInput")
    w_out_d = nc.dram_tensor("w_out", [128, 4, DM], f16, kind="ExternalInput")
    w_w1_d = nc.dram_tensor("w_w1", [DM, DFF], f16, kind="ExternalInput")
    w_w2_d = nc.dram_tensor("w_w2", [128, 8, DM], f16, kind="ExternalInput")
    w_cw_d = nc.dram_tensor("w_cw", [128, 4, 4], f32, kind="ExternalInput")
    w_cb_d = nc.dram_tensor("w_cb", [128, 4], f32, kind="ExternalInput")
    w_dtb_d = nc.dram_tensor("w_dtb", [128, 4], f32, kind="ExternalInput")
    w_D_d = nc.dram_tensor("w_D", [128, 4], f32, kind="ExternalInput")
    w_nrm_d = nc.dram_tensor("w_nrm", [128, 2], f32, kind="ExternalInput")
    w_b1_d = nc.dram_tensor("w_b1", [128, 8], f32, kind="ExternalInput")
    w_b2_d = nc.dram_tensor("w_b2", [128, 2], f32, kind="ExternalInput")
    yout_d = nc.dram_tensor("yout", [DM, T], f32, kind="ExternalOutput")

    with tile.TileContext(nc) as tc, ExitStack() as ctx:
        wp = ctx.enter_context(tc.tile_pool(name="wp", bufs=1))
        sp = ctx.enter_context(tc.tile_pool(name="sp", bufs=1))
        gp = ctx.enter_context(tc.tile_pool(name="gp", bufs=2))
        psp = ctx.enter_context(tc.tile_pool(name="psp", bufs=4, space="PSUM"))
        psx = ctx.enter_context(tc.tile_pool(name="psx", bufs=2, space="PSUM"))
        gp1 = ctx.enter_context(tc.tile_pool(name="gp1", bufs=1))
        drp = ctx.enter_context(tc.tile_pool(name="drp", bufs=1, space="DRAM"))

        # ---- load weights ----
        def wload(name, shape, dtype, dram, split=None):
            t = wp.tile(shape, dtype, tag=name)
            if split is None:
                nc.sync.dma_start(t[:], dram[:])
            else:
                for k in range(shape[1]):
                    nc.sync.dma_start(t[:, k, :], dram[k * 128:(k + 1) * 128, :])
            return t

        w_in = wload("w_in", [128, 2, DFF], f16, w_in_d, split=True)
        w_xp = wload("w_xp", [128, 4, 48], f16, w_xp_d)
        w_dtw = wload("w_dtw", [DTR, DI], f16, w_dtw_d)
        w_out = wload("w_out", [128, 4, DM], f16, w_out_d)
        w_w1 = wload("w_w1", [128, 2, DFF], f16, w_w1_d, split=True)
        w_w2 = wload("w_w2", [128, 8, DM], f16, w_w2_d)
        w_cw = wload("w_cw", [128, 4, 4], f32, w_cw_d)
        w_cb = wload("w_cb", [128, 4], f32, w_cb_d)
        w_dtb = wload("w_dtb", [128, 4], f32, w_dtb_d)
        w_D = wload("w_D", [128, 4], f32, w_D_d)
        w_nrm = wload("w_nrm", [128, 2], f32, w_nrm_d)
        w_b1 = wload("w_b1", [128, 8], f32, w_b1_d)
        w_b2 = wload("w_b2", [128, 2], f32, w_b2_d)
        ones_n = wp.tile([128, 128], f16, tag="ones_n")
        nc.gpsimd.memset(ones_n[:], 1.0 / DM)
        epsb = wp.tile([128, 1], f32, tag="epsb")
        nc.gpsimd.memset(epsb[:], 1e-5)
        oneb = wp.tile([128, 1], f32, tag="oneb")
        nc.gpsimd.memset(oneb[:], 1.0)

        # ---- persistent streams ----
        xin = sp.tile([128, 2, T], f16, tag="xin")
        nc.sync.dma_start(xin[:, 0, :], xin_d[0:128, :])
        nc.sync.dma_start(xin[:, 1, :], xin_d[128:256, :])
        u = sp.tile([128, 4, T], f16, tag="u")        # silu(conv(xc))
        zs = sp.tile([128, 4, T], f16, tag="zs")      # silu(z)
        dt = sp.tile([128, 4, T], f16, tag="dt")      # softplus dt
        ygz = sp.tile([128, 4, T], f16, tag="ygz")    # gated scan out
        bc_sb = sp.tile([48, T], f16, tag="bc")       # x_proj out (dtlr,B,C)
        xc_pre = sp.tile([128, 4, T], f16, tag="xc")  # pre-conv xc
        bc_dr = drp.tile([48, T], f16, tag="bcd")     # DRAM bounce for B/C

        NCH = T // CH

        # ================= phase A: in_proj, conv, x_proj, dt =============
        for ch in range(NCH):
            cs = slice(ch * CH, (ch + 1) * CH)
            for mt in range(8):
                ps = psp.tile([128, CH], f32, tag="mm")
                for kt in range(2):
                    nc.tensor.matmul(
                        ps[:], w_in[:, kt, mt * 128:(mt + 1) * 128],
                        xin[:, kt, cs],
                        start=(kt == 0), stop=(kt == 1))
                if mt < 4:   # xc half
                    nc.scalar.activation(xc_pre[:, mt, cs], ps[:], AF.Copy)
                else:        # z half: zs = z*sigmoid(z)
                    sg = gp.tile([128, CH], f16, tag="sg")
                    nc.scalar.activation(sg[:], ps[:], AF.Sigmoid)
                    nc.vector.tensor_tensor(zs[:, mt - 4, cs], ps[:], sg[:], MUL)

        # conv (depthwise, causal, k=4) + silu -> u
        for g in range(4):
            cv = gp.tile([128, T], f16, tag="ga")
            xg = xc_pre[:, g, :]
            # main taps over flat token axis (wrong at seq starts; fixed below)
            nc.vector.tensor_scalar(cv[:, 0:T], xg[:, 0:T],
                                    w_cw[:, g, 3:4], None, MUL)
            for k in (2, 1, 0):
                nc.vector.scalar_tensor_tensor(
                    cv[:, 3 - k:T], xg[:, 0:T - 3 + k], w_cw[:, g, k:k + 1],
                    cv[:, 3 - k:T], MUL, ADD)
            # fix first 3 cols of each seq: out[j] = sum_{i<=j} w[3-j+i]*x[i]
            xs = xc_pre[:, g, :].rearrange("p (s l) -> p s l", l=L)
            cs3 = cv[:].rearrange("p (s l) -> p s l", l=L)
            for j in range(3):
                nc.vector.tensor_scalar(
                    cs3[:, :, j:j + 1], xs[:, :, 0:1],
                    w_cw[:, g, 3 - j:4 - j], None, MUL)
                for i in range(1, j + 1):
                    nc.vector.scalar_tensor_tensor(
                        cs3[:, :, j:j + 1], xs[:, :, i:i + 1],
                        w_cw[:, g, 3 - j + i:4 - j + i], cs3[:, :, j:j + 1],
                        MUL, ADD)
            # u = t*sigmoid(t), t = cv + conv_b
            cb = gp.tile([128, T], f16, tag="gb")
            nc.vector.tensor_scalar(cb[:], cv[:], w_cb[:, g:g + 1], None, ADD)
            sgc = gp.tile([128, T], f16, tag="gh")
            nc.scalar.activation(sgc[:], cb[:], AF.Sigmoid)
            nc.vector.tensor_tensor(u[:, g, :], cb[:], sgc[:], MUL)

        # x_proj: dbl = u @ x_projT -> [48, T] (dtlr 0:16, B 16:32, C 32:48)
        for ch in range(NCH):
            cs = slice(ch * CH, (ch + 1) * CH)
            px = psx.tile([48, CH], f32, tag="xp")
            for kt in range(4):
                nc.tensor.matmul(px[:], w_xp[:, kt, :],
                                 u[:, kt, cs], start=(kt == 0), stop=(kt == 3))
            nc.vector.tensor_copy(bc_sb[:, cs], px[:])
        # bounce B/C rows to DRAM for partition-replication reads
        nc.sync.dma_start(bc_dr[:], bc_sb[:])

        # dt path: v = dtlr @ dt_wT + dt_b ; dt = Ln(Exp(v)+1)
        for ch in range(NCH):
            cs = slice(ch * CH, (ch + 1) * CH)
            for mt in range(4):
                pd = psp.tile([128, CH], f32, tag="mm")
                nc.tensor.matmul(pd[:], w_dtw[:, mt * 128:(mt + 1) * 128],
                                 bc_sb[0:16, cs], start=True, stop=True)
                ev = gp.tile([128, CH], f16, tag="sg")
                nc.scalar.activation(ev[:], pd[:], AF.Exp, bias=w_dtb[:, mt:mt + 1])
                nc.scalar.activation(dt[:, mt, cs], ev[:], AF.Ln, bias=oneb[:])

        # ================= phase B: scan grid + gate =====================
        for tc_i in range(T // TCH):
            ts = slice(tc_i * TCH, (tc_i + 1) * TCH)
            wdt = gp1.tile([128, 4, TCH], f16, tag="wdt")
            for g in range(4):
                nc.vector.tensor_tensor(wdt[:, g, :], dt[:, g, ts],
                                        u[:, g, ts], MUL)
            for nb in range(NST // NB):
                # B_rep/C_rep [128, NB, TCH] replicated from DRAM rows,
                # shared across the 4 d-groups
                B_rep = gp.tile([128, NB, TCH], f16, tag="gB")
                C_rep = gp.tile([128, NB, TCH], f16, tag="gB")
                for j in range(NB):
                    n_ = nb * NB + j
                    nc.sync.dma_start(
                        B_rep[:, j, :],
                        bc_dr[16 + n_:17 + n_, ts].broadcast_to([128, TCH]))
                    nc.sync.dma_start(
                        C_rep[:, j, :],
                        bc_dr[32 + n_:33 + n_, ts].broadcast_to([128, TCH]))
                for g in range(4):
                    # a = exp(-n*dt)
                    a = gp.tile([128, NB, TCH], f16, tag="ga")
                    for j in range(NB):
                        n_val = nb * NB + j + 1
                        nc.scalar.activation(a[:, j, :], dt[:, g, ts], AF.Exp,
                                             scale=-float(n_val))
                    av = a[:].rearrange("p n (s l) -> p n s l", l=L)
                    nc.gpsimd.memset(av[:, :, :, 0:1], 0.0)
                    # b = wdt (x) B
                    b = gp.tile([128, NB, TCH], f16, tag="gb")
                    nc.vector.tensor_tensor(
                        b[:],
                        wdt[:, g, :].unsqueeze(1).broadcast_to([128, NB, TCH]),
                        B_rep[:], MUL)
                    # h = scan(a, b)
                    h = gp.tile([128, NB, TCH], f16, tag="gh")
                    nc.vector.tensor_tensor_scan(
                        h[:].rearrange("p n t -> p (n t)"),
                        a[:].rearrange("p n t -> p (n t)"),
                        b[:].rearrange("p n t -> p (n t)"),
                        0.0, MUL, ADD)
                    # p = h*C, partial tree accumulated into ygz
                    p = gp.tile([128, NB, TCH], f16, tag="ga")
                    nc.vector.tensor_tensor(p[:], h[:], C_rep[:], MUL)
                    s2 = gp.tile([128, 2, TCH], f16, tag="gs")
                    nc.vector.tensor_tensor(s2[:], p[:, 0:2, :], p[:, 2:4, :], ADD)
                    if nb == 0:
                        nc.vector.tensor_tensor(ygz[:, g, ts], s2[:, 0, :],
                                                s2[:, 1, :], ADD)
                    else:
                        s1 = gp.tile([128, TCH], f16, tag="gs1")
                        nc.vector.tensor_tensor(s1[:], s2[:, 0, :],
                                                s2[:, 1, :], ADD)
                        nc.vector.tensor_tensor(ygz[:, g, ts], s1[:],
                                                ygz[:, g, ts], ADD)
            # gate: ygz = (ygz + D*u) * zs
            for g in range(4):
                t1 = gp.tile([128, TCH], f16, tag="gs1")
                nc.vector.scalar_tensor_tensor(t1[:], u[:, g, ts],
                                               w_D[:, g:g + 1],
                                               ygz[:, g, ts], MUL, ADD)
                nc.vector.tensor_tensor(ygz[:, g, ts], t1[:], zs[:, g, ts], MUL)

        # ================= phase C: out_proj, norms, MLP =================
        for ch in range(NCH):
            cs = slice(ch * CH, (ch + 1) * CH)
            # out_proj + residual -> res1 [256, CH] f32
            res1 = gp.tile([128, 2, CH], f32, tag="gb")
            for mt in range(2):
                po = psp.tile([128, CH], f32, tag="mm")
                for kt in range(4):
                    nc.tensor.matmul(po[:], w_out[:, kt, mt * 128:(mt + 1) * 128],
                                     ygz[:, kt, cs], start=(kt == 0), stop=(kt == 3))
                nc.vector.tensor_tensor(res1[:, mt, :], po[:], xin[:, mt, cs], ADD)
            # rmsnorm -> z1 f16 (includes weight)
            z1 = gp.tile([128, 2, CH], f16, tag="gh")
            _rmsnorm(nc, gp, psx, res1, z1, w_nrm, ones_n, epsb, AF, MUL, f16, f32)
            # W1 @ z1 -> gelu -> mh
            mh = gp.tile([128, 8, CH], f16, tag="gB")
            for mt in range(8):
                pm = psp.tile([128, CH], f32, tag="mm")
                for kt in range(2):
                    nc.tensor.matmul(pm[:], w_w1[:, kt, mt * 128:(mt + 1) * 128],
                                     z1[:, kt, :], start=(kt == 0), stop=(kt == 1))
                if sim_gelu:
                    sgm = gp.tile([128, CH], f16, tag="sg")
                    nc.scalar.activation(sgm[:], pm[:], AF.Sigmoid, scale=1.702,
                                         bias=w_b1[:, mt:mt + 1])
                    ptmp = gp.tile([128, CH], f32, tag="srt")
                    nc.scalar.activation(ptmp[:], pm[:], AF.Identity,
                                         bias=w_b1[:, mt:mt + 1])
                    nc.vector.tensor_tensor(mh[:, mt, :], ptmp[:], sgm[:], MUL)
                else:
                    nc.scalar.activation(mh[:, mt, :], pm[:], AF.Gelu,
                                         bias=w_b1[:, mt:mt + 1])
            # W2 @ mh + b2 + z1 -> res2
            res2 = gp.tile([128, 2, CH], f32, tag="gb")
            for mt in range(2):
                pw = psp.tile([128, CH], f32, tag="mm")
                for kt in range(8):
                    nc.tensor.matmul(pw[:], w_w2[:, kt, mt * 128:(mt + 1) * 128],
                                     mh[:, kt, :], start=(kt == 0), stop=(kt == 7))
                nc.vector.scalar_tensor_tensor(res2[:, mt, :], pw[:],
                                               w_b2[:, mt:mt + 1],
                                               z1[:, mt, :], ADD, ADD)
            # rmsnorm2 -> yout (f32)
            z2 = gp.tile([128, 2, CH], f32, tag="gh")
            _rmsnorm(nc, gp, psx, res2, z2, w_nrm, ones_n, epsb, AF, MUL, f16, f32)
            nc.sync.dma_start(yout_d[0:128, cs], z2[:, 0, :])
            nc.sync.dma_start(yout_d[128:256, cs], z2[:, 1, :])

    nc.compile()
    return nc


def _rmsnorm(nc, gp, psx, res, zout, w_nrm, ones_n, epsb, AF, MUL, f16, f32):
    """zout = res * w * rsqrt(mean(res^2 over 256 feats) + eps).

    Partition reduction via ones-matmul (lhsT = 1/256) which also broadcasts
    the mean to every output partition.
    """
    CHn = res.shape[-1]
    sq = gp.tile([128, 2, CHn], f16, tag="gs")
    nc.scalar.activation(sq[:, 0, :], res[:, 0, :], AF.Square)
    nc.scalar.activation(sq[:, 1, :], res[:, 1, :], AF.Square)
    pn = psx.tile([128, CHn], f32, tag="xp")
    for kt in range(2):
        nc.tensor.matmul(pn[:], ones_n[:], sq[:, kt, :],
                         start=(kt == 0), stop=(kt == 1))
    srt = gp.tile([128, CHn], f32, tag="srt")
    nc.scalar.activation(srt[:], pn[:], AF.Sqrt, bias=epsb[:])
    inv = gp.tile([128, CHn], f32, tag="inv")
    nc.vector.reciprocal(inv[:], srt[:])
    for mt in range(2):
        nc.vector.scalar_tensor_tensor(zout[:, mt, :], res[:, mt, :],
                                       w_nrm[:, mt:mt + 1], inv[:], MUL, MUL)


# ---------------------------------------------------------------- host ----
def _prep_mammlp_weights(pm):
    """pm: {'mamba': {...}, 'W1','b1','W2','b2','norm1'} -> in_map arrays."""
    f16 = np.float16
    f32 = np.float32
    p = pm["mamba"]
    W = {}
    W["w_in"] = np.ascontiguousarray(np.asarray(p["in_proj"], f32).T).astype(f16)
    W["w_xp"] = np.ascontiguousarray(
        np.asarray(p["x_proj"], f32).T.reshape(4, 128, 48).transpose(1, 0, 2)
    ).astype(f16)
    W["w_dtw"] = np.ascontiguousarray(np.asarray(p["dt_w"], f32).T).astype(f16)
    W["w_out"] = np.ascontiguousarray(
        np.asarray(p["out_proj"], f32).T.reshape(4, 128, 256).transpose(1, 0, 2)
    ).astype(f16)
    W["w_cw"] = np.ascontiguousarray(
        np.asarray(p["conv_w"], f32).reshape(4, 128, 4).transpose(1, 0, 2))
    W["w_cb"] = np.asarray(p["conv_b"], f32).reshape(4, 128).T.copy()
    W["w_dtb"] = np.asarray(p["dt_b"], f32).reshape(4, 128).T.copy()
    W["w_D"] = np.asarray(p["D"], f32).reshape(4, 128).T.copy()
    W["w_w1"] = np.ascontiguousarray(np.asarray(pm["W1"], f32).T).astype(f16)
    W["w_w2"] = np.ascontiguousarray(
        np.asarray(pm["W2"], f32).T.reshape(8, 128, 256).transpose(1, 0, 2)
    ).astype(f16)
    W["w_b1"] = np.asarray(pm["b1"], f32).reshape(8, 128).T.copy()
    W["w_b2"] = np.asarray(pm["b2"], f32).reshape(2, 128).T.copy()
    W["w_nrm"] = np.asarray(pm["norm1"], f32).reshape(2, 128).T.copy()
    return W


def _time_inputs(xf):
    """xf [4,32,128,256] -> 8x [256, 2048] f16 (16 (b,tsd)-seqs, t=cyc)."""
    seqs = xf.reshape(B_ * TSD, CYC, DM)
    return [np.ascontiguousarray(
        seqs[c * 16:(c + 1) * 16].reshape(T, DM).T.astype(np.float16))
        for c in range(NCORES)]


def _time_outputs(res):
    seqs = np.empty((B_ * TSD, CYC, DM), np.float32)
    for c in range(NCORES):
        seqs[c * 16:(c + 1) * 16] = res[c].T.reshape(16, CYC, DM)
    return seqs.reshape(B_, TSD, CYC, DM)


def _dim_inputs(xf):
    """xf [4,32,128,256] -> 8x [256, 2048] f16 (64 (b,cyc)-seqs, t=tsd)."""
    seqs = xf.transpose(0, 2, 1, 3).reshape(B_ * CYC, TSD, DM)
    return [np.ascontiguousarray(
        seqs[c * 64:(c + 1) * 64].reshape(T, DM).T.astype(np.float16))
        for c in range(NCORES)]


def _dim_outputs(res):
    seqs = np.empty((B_ * CYC, TSD, DM), np.float32)
    for c in range(NCORES):
        seqs[c * 64:(c + 1) * 64] = res[c].T.reshape(64, TSD, DM)
    return seqs.reshape(B_, CYC, TSD, DM).transpose(0, 2, 1, 3)


_PROGRAMS = {}


def _get_program(kind):
    if kind not in _PROGRAMS:
        if kind == "time":
            _PROGRAMS[kind] = build_program(16, 128, sim_gelu=SIM_GELU)
        else:
            _PROGRAMS[kind] = build_program(64, 32, sim_gelu=SIM_GELU)
    return _PROGRAMS[kind]


LAUNCH_STATS = []


def _launch(kind, xcores, wmap):
    """xcores: 8x [256,2048] f16; wmap: weight arrays (shared by cores)."""
    from concourse import bass_utils
    from concourse.bass_interp import get_hw_module
    nc = _get_program(kind)
    in_maps = [{"xin": xcores[c], **wmap} for c in range(NCORES)]
    trace = bool(int(os.environ.get("KBENCH_TRACE", "0")))
    old_m = nc.m
    nc.m = get_hw_module(nc.m)
    try:
        r = bass_utils.run_bass_kernel_spmd(nc, in_maps,
                                            core_ids=list(range(NCORES)),
                                            trace=trace)
    finally:
        nc.m = old_m
    LAUNCH_STATS.append({"kind": kind, "exec_time_ns": r.exec_time_ns})
    return [r.results[c]["yout"] for c in range(NCORES)]


def _mammlp_dev(kind, xf, wmap):
    if kind == "time":
        return _time_outputs(_launch("time", _time_inputs(xf), wmap))
    return _dim_outputs(_launch("dim", _dim_inputs(xf), wmap))


def _fdse_dev(xf, pblk):
    t = _mammlp_dev("time", xf, _prep_mammlp_weights(pblk["time"]))
    return _mammlp_dev("dim", t, _prep_mammlp_weights(pblk["dim"]))


def _noise_coeffs(std):
    ac = np.cumprod(1.0 - np.linspace(0.0, std, 1000))
    return np.float32(np.sqrt(ac[-1])), np.float32(np.sqrt(1.0 - ac[-1]))


_NOISE_SRC = """
import os
os.environ["JAX_PLATFORMS"] = "cpu"
import sys, numpy as np, jax
key = jax.random.key(42)
ns = [np.asarray(jax.random.normal(jax.random.fold_in(key, i),
                                   (4, 32, 128, 256), np.float32))
      for i in range(3)]
np.save(sys.argv[1], np.stack(ns))
"""


def _compute_noises():
    """Bit-exact jax-cpu threefry noise, computed in a CPU-pinned subprocess
    (this process's jax may be pinned to the accelerator platform)."""
    import jax

    try:
        cpu = jax.devices("cpu")[0]
        with jax.default_device(cpu):
            key = jax.random.key(42)
            return [np.asarray(jax.random.normal(jax.random.fold_in(key, i),
                                                 (B_, TSD, CYC, DM),
                                                 np.float32))
                    for i in range(3)]
    except Exception:
        pass
    import subprocess, tempfile
    with tempfile.TemporaryDirectory() as td:
        path = os.path.join(td, "n.npy")
        subprocess.run([sys.executable, "-c", _NOISE_SRC, path], check=True)
        arr = np.load(path)
    return [arr[0], arr[1], arr[2]]


def kernel(x, params):
    x = np.asarray(x, np.float32)
    tonp = lambda d: {k: (tonp(v) if isinstance(v, dict) else np.asarray(v))
                      for k, v in d.items()}
    params = tonp(params)

    x1 = _fdse_dev(x, params["b1"])

    noises = _compute_noises()

    outs = [x, x1]
    for i, (std, blk) in enumerate([(0.2, "b2"), (0.01, "b3"), (0.001, "b4")]):
        sa, so = _noise_coeffs(std)
        yin = sa * x1 + so * noises[i]
        outs.append(_fdse_dev(yin, params[blk]))
    return np.stack(outs).astype(np.float32)
